# revision 9
# baseline (speedup 1.0000x reference)
"""Distributed Trainium2 (8 NeuronCores) attention-head kernel.

Problem: single attention head with projections.
  q = Q @ Wq.T + bq ; k = K @ Wk.T + bk ; v = V @ Wv.T + bv
  x = (q @ k.T) / sqrt(64) ; x = x*m - 1e9*(1-m) ; p = softmax(x)
  y = p @ v
Shapes: Q/K/V [2, 4096, 1024] f32, mask [2, 4096, 4096] int32 -> y [2, 4096, 64] f32.

Strategy (8 cores): shard queries 8-ways (2 batches x 4 query-chunks of 1024
rows).  K/V are replicated within each 4-core batch group (collective_compute
has ~100us fixed overhead on this fleet; bf16 replication is cheaper).  The
host reshards into matmul-native transposed layouts (contraction dim on
partitions), packed partition-major so every DMA is a full-width [128, W] tile
with >=8KB contiguous per-partition rows (measured ~430GB/s vs 41GB/s at 1KB
rows).  K/V are additionally packed s-group-major so each 2MB group load is
immediately projectable (the dm-contraction needs all 8 dm-chunks of a group).
Q/K/V/W/mask are cast to bf16 (mask 0/1 is exact; bf16 mask keeps the DVE
multiply in its fast mode); softmax is computed as p=exp(x/8)*m,
y=(p@v)/sum(p) - algebraically identical to the reference's masked softmax
(no fully-masked rows exist).

The dk=64 contraction only fills half the PE array, so score matmuls run
packed 2-up: kT/qT are mirrored onto partitions 64-127 (SBUF->SBUF DMA) and
even/odd s-chunks execute concurrently in PE row-groups (0,0)/(64,0).

Per-core pipeline (layouts [partitions, free]):
  qT[64,1024] = sum_j WqT[j].T @ QT[j]        (dm-chunk j, PSUM accumulate)
  per s-group g (4 x 1024): kT[:,g] = proj(K), vT[:,g] = proj(V),
     v_aug[s,65] = [v | 1] via PE transposes of vT
     per s-chunk pair: sT = kT_chunk.T @ qT (2-up) ; p = exp(sT/8) * maskT
                       yT[65,1024] += v_aug_chunk.T @ p  (PSUM accum)
  y[q,65] = transpose(yT); out = y[:, :64] / y[:, 64:65]
"""

import numpy as np
import ml_dtypes

import concourse.bass as bass
import concourse.mybir as mybir
import concourse.tile as tile
from concourse import bacc
from concourse.bass_utils import run_bass_kernel_spmd
from concourse.masks import make_identity

B, S, DM, DK = 2, 4096, 1024, 64
N_CORES = 8
GROUP = 4            # cores per batch
SQ = S // GROUP      # query rows per core (1024)
NDM = DM // 128      # dm chunks (8)
NSG = 4              # s groups (1024 rows each)
SG = S // NSG        # 1024
NSC_G = SG // 128    # s chunks per group (8)

F32 = mybir.dt.float32
BF16 = mybir.dt.bfloat16

_last_results = None


def _build():
    nc = bacc.Bacc(None, target_bir_lowering=False)

    qt_e = nc.declare_dram_parameter("qt", [128, NDM * SQ], BF16, isOutput=False)
    kt_e = nc.declare_dram_parameter("kt", [128, NDM * S], BF16, isOutput=False)
    vt_e = nc.declare_dram_parameter("vt", [128, NDM * S], BF16, isOutput=False)
    mt_e = nc.declare_dram_parameter("mt", [128, S * SQ // 128], BF16, isOutput=False)
    w_e = nc.declare_dram_parameter("wqkv", [128, 3 * NDM * DK], BF16, isOutput=False)
    b_e = nc.declare_dram_parameter("bqkv", [DK, 3], F32, isOutput=False)
    out_e = nc.declare_dram_parameter("out", [128, (SQ // 128) * DK], F32, isOutput=True)

    GW = NDM * SG    # columns per kt/vt group slice (8192)
    MW = NSC_G * SQ  # columns per mask group slice (8192)

    with tile.TileContext(nc) as tc:
        with (
            tc.tile_pool(name="const", bufs=1) as cpool,
            tc.tile_pool(name="mask", bufs=NSG) as mpool,
            tc.tile_pool(name="kin", bufs=3) as kpool,
            tc.tile_pool(name="big", bufs=1) as bigpool,
            tc.tile_pool(name="pp", bufs=6) as ppool,
            tc.tile_pool(name="small", bufs=4) as spool,
            tc.tile_pool(name="psum_s", bufs=3, space="PSUM") as psT,
            tc.tile_pool(name="psum_y", bufs=1, space="PSUM") as pyT,
        ):
            # ---- loads, in consumption order ----
            qt_sb = cpool.tile([128, NDM * SQ], BF16, tag="qt")
            nc.sync.dma_start(qt_sb[:], qt_e[:])
            w_sb = cpool.tile([128, 3 * NDM * DK], BF16, tag="w")
            nc.sync.dma_start(w_sb[:], w_e[:])
            b_sb = cpool.tile([DK, 3], F32, tag="b")
            nc.sync.dma_start(b_sb[:], b_e[:])

            kt_t, vt_t, mq_t = {}, {}, {}
            for g in range(NSG):
                kt_t[g] = kpool.tile([128, GW], BF16, tag="xin", name=f"kt_g{g}")
                nc.sync.dma_start(kt_t[g][:], kt_e[:, g * GW:(g + 1) * GW])
                if g == 0:
                    mq_t[0] = mpool.tile([128, MW], BF16, tag="mt", name="mq_g0")
                    nc.sync.dma_start(mq_t[0][:], mt_e[:, 0:MW])
                vt_t[g] = kpool.tile([128, GW], BF16, tag="xin", name=f"vt_g{g}")
                nc.sync.dma_start(vt_t[g][:], vt_e[:, g * GW:(g + 1) * GW])
                if g in (1, 2):
                    mq_t[g] = mpool.tile([128, MW], BF16, tag="mt", name=f"mq_g{g}")
                    nc.sync.dma_start(mq_t[g][:], mt_e[:, g * MW:(g + 1) * MW])
            mq_t[3] = mpool.tile([128, MW], BF16, tag="mt", name="mq_g3")
            nc.sync.dma_start(mq_t[3][:], mt_e[:, 3 * MW:4 * MW])

            ident_bf = cpool.tile([128, 128], BF16, tag="ident_bf")
            make_identity(nc, ident_bf[:])
            ident_f32 = cpool.tile([128, 128], F32, tag="ident_f32")
            make_identity(nc, ident_f32[:])

            def wsl(which, j):  # weight chunk slice in w_sb
                return w_sb[:, (which * NDM + j) * DK:(which * NDM + j + 1) * DK]

            # ---- q projection: qT[64, 1024], mirrored to partitions 64-127 ----
            qT_sb = bigpool.tile([128, SQ], BF16, tag="qT")
            ps = psT.tile([DK, 1024], F32, tag="sT", name="ps_q")
            for j in range(NDM):
                for h in range(2):
                    c0 = j * SQ + h * 512
                    nc.tensor.matmul(
                        ps[:, h * 512:(h + 1) * 512],
                        lhsT=wsl(0, j), rhs=qt_sb[:, c0:c0 + 512],
                        start=(j == 0), stop=(j == NDM - 1),
                    )
            nc.vector.tensor_scalar_add(qT_sb[:DK, :], ps[:], b_sb[:, 0:1])
            nc.sync.dma_start(qT_sb[DK:128, :], qT_sb[:DK, :])

            kT_sb = bigpool.tile([128, S], BF16, tag="kT")
            vT_sb = bigpool.tile([DK, S], BF16, tag="vT")
            v_aug = bigpool.tile([128, S // 128 * 65], BF16, tag="vaug")
            nc.vector.memset(v_aug[:], 1.0)
            yT_ps = pyT.tile([65, SQ], F32, tag="yT")

            for g in range(NSG):
                # ---- k/v projections for this s-group ----
                for which, t, dst in ((1, kt_t[g], kT_sb), (2, vt_t[g], vT_sb)):
                    ps = psT.tile([DK, 1024], F32, tag="sT", name=f"ps_{which}_{g}")
                    for j in range(NDM):
                        for h in range(2):
                            c0 = j * SG + h * 512
                            nc.tensor.matmul(
                                ps[:, h * 512:(h + 1) * 512],
                                lhsT=wsl(which, j), rhs=t[:, c0:c0 + 512],
                                start=(j == 0), stop=(j == NDM - 1),
                            )
                    nc.vector.tensor_scalar_add(
                        dst[:DK, g * SG:(g + 1) * SG], ps[:], b_sb[:, which:which + 1]
                    )
                # mirror kT group to partitions 64-127 for 2-up scores
                nc.sync.dma_start(
                    kT_sb[DK:128, g * SG:(g + 1) * SG],
                    kT_sb[:DK, g * SG:(g + 1) * SG],
                )
                # ---- v_aug transposes for this group ----
                for jj in range(NSC_G):
                    sc = g * NSC_G + jj
                    pt = psT.tile([128, DK], BF16, tag="sT", name=f"pt_{sc}")
                    nc.tensor.transpose(
                        pt[:], vT_sb[:, sc * 128:(sc + 1) * 128],
                        ident_bf[:DK, :DK],
                    )
                    nc.vector.tensor_copy(v_aug[:, sc * 65:sc * 65 + DK], pt[:])
                # ---- main chunks of this group, scores packed 2-up ----
                for jj in range(0, NSC_G, 2):
                    sc0 = g * NSC_G + jj          # even chunk -> rows 0-63
                    sc1 = sc0 + 1                 # odd chunk  -> rows 64-127
                    sT0 = psT.tile([128, SQ], F32, tag="sT", name=f"sT_{sc0}")
                    sT1 = psT.tile([128, SQ], F32, tag="sT", name=f"sT_{sc1}")
                    for h in range(2):
                        hs = slice(h * 512, (h + 1) * 512)
                        nc.tensor.matmul(
                            sT0[:, hs],
                            lhsT=kT_sb[0:DK, sc0 * 128:(sc0 + 1) * 128],
                            rhs=qT_sb[0:DK, hs],
                            start=True, stop=True,
                        )
                        nc.tensor.matmul(
                            sT1[:, hs],
                            lhsT=kT_sb[DK:128, sc1 * 128:(sc1 + 1) * 128],
                            rhs=qT_sb[DK:128, hs],
                            start=True, stop=True,
                        )
                    for sc, sT in ((sc0, sT0), (sc1, sT1)):
                        jq = sc - g * NSC_G
                        p = ppool.tile([128, SQ], BF16, tag="p", name=f"p_{sc}")
                        nc.scalar.activation(
                            p[:], sT[:], mybir.ActivationFunctionType.Exp,
                            scale=0.125,
                        )
                        nc.vector.tensor_mul(
                            p[:], p[:], mq_t[g][:, jq * SQ:(jq + 1) * SQ]
                        )
                        for h in range(2):
                            hs = slice(h * 512, (h + 1) * 512)
                            nc.tensor.matmul(
                                yT_ps[:, hs],
                                lhsT=v_aug[:, sc * 65:(sc + 1) * 65],
                                rhs=p[:, hs],
                                start=(sc == 0), stop=(sc == S // 128 - 1),
                            )

            # ---- epilogue: y = transpose(yT); out = y[:, :64] / y[:, 64] ----
            yT_sb = bigpool.tile([65, SQ], F32, tag="yT_sb")
            nc.scalar.copy(yT_sb[:], yT_ps[:])
            y_all = bigpool.tile([128, (SQ // 128) * DK], F32, tag="y_all")
            for t in range(SQ // 128):
                yp = psT.tile([128, 65], F32, tag="sT", name=f"yp_{t}")
                nc.tensor.transpose(
                    yp[:], yT_sb[:, t * 128:(t + 1) * 128], ident_f32[:65, :65]
                )
                rcp = spool.tile([128, 1], F32, tag="rcp", name=f"rcp_{t}")
                nc.vector.reciprocal(rcp[:], yp[:, DK:DK + 1])
                nc.vector.tensor_scalar_mul(
                    y_all[:, t * DK:(t + 1) * DK], yp[:, :DK], rcp[:]
                )
            nc.sync.dma_start(out_e[:], y_all[:])

    nc.finalize()
    return nc


def _pack(at, w):
    """[R, W] -> [128, (R//128)*W]: row p gets rows {p, 128+p, ...}."""
    r = at.shape[0]
    return np.ascontiguousarray(
        at.reshape(r // 128, 128, w).transpose(1, 0, 2).reshape(128, -1)
    )


def _pack_groups(at):
    """KT/VT [1024, 4096] -> [128, 4*8*1024]: s-group-major partition packing.
    col ((g*8 + j)*1024 + s') on row p = at[j*128 + p, g*1024 + s']."""
    a = at.reshape(NDM, 128, NSG, SG)         # [j, p, g, s']
    return np.ascontiguousarray(
        a.transpose(1, 2, 0, 3).reshape(128, -1)
    )


def kernel(Q, K, V, mask, Wq, bq, Wk, bk, Wv, bv):
    global _last_results
    bf16 = ml_dtypes.bfloat16

    w_p = np.concatenate(
        [_pack(W.T.astype(bf16), DK) for W in (Wq, Wk, Wv)], axis=1
    )
    b_p = np.ascontiguousarray(
        np.stack([bq, bk, bv], axis=1).astype(np.float32)
    )

    kt_b = [_pack_groups(K[b].T.astype(bf16)) for b in range(B)]
    vt_b = [_pack_groups(V[b].T.astype(bf16)) for b in range(B)]

    in_maps = []
    for c in range(N_CORES):
        b, i = divmod(c, GROUP)
        rows = slice(i * SQ, (i + 1) * SQ)
        in_maps.append({
            "qt": _pack(np.ascontiguousarray(Q[b, rows, :].T).astype(bf16), SQ),
            "kt": kt_b[b],
            "vt": vt_b[b],
            "mt": _pack(np.ascontiguousarray(mask[b, rows, :].T).astype(bf16), SQ),
            "wqkv": w_p,
            "bqkv": b_p,
        })

    nc = _build()
    res = run_bass_kernel_spmd(nc, in_maps, core_ids=list(range(N_CORES)))
    _last_results = res

    out = np.empty((B, S, DK), dtype=np.float32)
    for c in range(N_CORES):
        b, i = divmod(c, GROUP)
        y = res.results[c]["out"].reshape(128, SQ // 128, DK).transpose(1, 0, 2)
        out[b, i * SQ:(i + 1) * SQ, :] = y.reshape(SQ, DK)
    return out
